# revision 5
# baseline (speedup 1.0000x reference)
"""Bass/Trainium2 kernel for nn_Block_60224031424641 (SegNeXt MSCAN block).

Reference computation (per image, NCHW, C=64, H=W=256):
  n1   = BN(x)                                (eval-mode batchnorm)
  c55  = dw5x5(n1) + bb55
  c17  = dw7x1(dw1x7(n1) + b17a) + b17b       (and 11, 21 analogues)
  mix  = 1x1(c55 + c17 + c111 + c211) + b11
  x    = x + ls1 * (mix * n1)
  n2   = BN2(x)
  hdn  = gelu(dw3x3(1x1(n2) + fb1) + fbdw)
  out  = x + ls2 * (1x1(hdn) + fb2)

Sharding: 8 cores = (batch 4) x (image h-half 2), pure data parallel with
host-provided halo rows (no cross-core communication).

Per-core layout: SBUF partitions = (s, c) where s in {0,1} is a further
h-quarter split and c the 64 channels; free dim = (rows, w).  All conv
shifts are free-dim offsets; per-channel conv taps are per-partition
scalars consumed by DVE scalar_tensor_tensor ops.  1x1 convs run on the
tensor engine with block-diagonal (over s) lhsT weights; the FFN's 3x3
depthwise conv is folded into the fw1 matmul (9 accumulating matmuls
with shifted rhs).  Image-boundary zero-padding is handled by per-core
bias columns (out-of-image regions get a zeroed bias so BN/bias never
re-introduces nonzeros where the reference zero-pads).
"""

import numpy as np
import ml_dtypes

import concourse.bass as bass
import concourse.bacc as bacc
import concourse.mybir as mybir
import concourse.tile as tile
from concourse.bass_utils import run_bass_kernel_spmd

F32 = mybir.dt.float32
BF16 = mybir.dt.bfloat16
AO = mybir.AluOpType
AF = mybir.ActivationFunctionType
BF = ml_dtypes.bfloat16

# ---------------- geometry ----------------
C = 64          # channels
W = 256         # image width
HALO = 11       # input halo rows each side (10 conv + 1 ffn)
SR = 128 + 2 * HALO          # 150 slice rows per core
LR = 64 + 2 * HALO           # 86 rows per (s) half
WP = 276        # n1 padded width (10 each side)
P1 = 10         # n1 left pad
BR = 66         # bsum / attn / n2 rows (out-relative [-1, 65))
NW = 258        # n2 padded width (1 each side)
RB = HALO       # local row of first out row (11)
CH = 8          # ffn chunk rows
NCH = 8         # ffn chunks (8*8 = 64 out rows per half)
EPS = 1e-5

# ---------------- cvec column registry ----------------
_COLS: dict[str, int] = {}


def _col(name: str) -> int:
    if name not in _COLS:
        _COLS[name] = len(_COLS)
    return _COLS[name]


def _build_cols():
    for n in ("s1", "t1", "t1top", "t1bot", "s2", "t2", "t2top", "t2bot",
              "b11p", "ls1", "ls2", "fb2p",
              "b17a", "b17at", "b17ab",
              "b111a", "b111at", "b111ab",
              "b211a", "b211at", "b211ab"):
        _col(n)
    for t in range(4):
        _col(f"fb1p{t}")
        _col(f"fb1e{t}")
        _col(f"fb1f{t}")
    for dh in range(5):
        for dw in range(5):
            _col(f"w55_{dh}_{dw}")
    for dw in range(7):
        _col(f"w17a_{dw}")
    for dh in range(7):
        _col(f"w17b_{dh}")
    for dw in range(11):
        _col(f"w111a_{dw}")
    for dh in range(11):
        _col(f"w111b_{dh}")
    for dw in range(21):
        _col(f"w211a_{dw}")
    for dh in range(21):
        _col(f"w211b_{dh}")


_build_cols()
NCOL = len(_COLS)

# tabs (bf16 matmul weight tables): 36 fw1-fold blocks, 4 fw2 blocks, 1 w11
TB_FW1 = 0                # + (ti*9 + d) * 128, d = dh*3+dw
TB_FW2 = 36 * 128         # + ti * 128
TB_W11 = 40 * 128
TBN = 41 * 128


# ---------------- device kernel ----------------
def build_nc():
    nc = bacc.Bacc("TRN2")
    x_d = nc.dram_tensor("xs", [128, LR, W], F32, kind="ExternalInput")
    cv_d = nc.dram_tensor("cvec", [128, NCOL], F32, kind="ExternalInput")
    tb_d = nc.dram_tensor("tabs", [128, TBN], BF16, kind="ExternalInput")
    o_d = nc.dram_tensor("out", [128, 64, W], F32, kind="ExternalOutput")

    with tile.TileContext(nc) as tc:
        with tc.tile_pool(name="P", bufs=1) as P, \
             tc.tile_pool(name="XST", bufs=2) as XST, \
             tc.tile_pool(name="PS", bufs=2, space="PSUM") as PS:

            cv = P.tile([128, NCOL], F32, tag="cv")
            nc.sync.dma_start(out=cv[:], in_=cv_d[:])
            tb = P.tile([128, TBN], BF16, tag="tb")
            nc.sync.dma_start(out=tb[:], in_=tb_d[:])

            def col(name, p0=0, p1=128):
                i = _COLS[name]
                return cv[p0:p1, i:i + 1]

            def blk(i):
                return tb[:, i * 128:(i + 1) * 128]

            # ---- n1 = BN1(x), streamed, with boundary-masked bias ----
            n1 = P.tile([128, LR, WP], BF16, tag="n1")
            nc.gpsimd.memset(n1[:], 0.0)
            # region table: (p0, p1, r0, r1, biascol); rows are local [0, 86)
            bn1_regions = [
                (0, 64, 0, HALO, "t1top"),
                (0, 64, HALO, LR, "t1"),
                (64, 128, 0, LR - HALO, "t1"),
                (64, 128, LR - HALO, LR, "t1bot"),
            ]
            nchunk = (LR + CH - 1) // CH
            for k in range(nchunk):
                r0, r1 = k * CH, min((k + 1) * CH, LR)
                xst = XST.tile([128, CH, W], F32, tag="xst")
                nc.sync.dma_start(out=xst[:, :r1 - r0, :], in_=x_d[:, r0:r1, :])
                for (p0, p1, g0, g1, bc) in bn1_regions:
                    a0, a1 = max(g0, r0), min(g1, r1)
                    if a0 >= a1:
                        continue
                    nc.scalar.activation(
                        out=n1[p0:p1, a0:a1, P1:P1 + W],
                        in_=xst[p0:p1, a0 - r0:a1 - r0, :],
                        func=AF.Identity,
                        bias=col(bc, p0, p1),
                        scale=col("s1", p0, p1),
                    )

            # ---- depthwise conv stack -> bsum (bf16) ----
            bsum = P.tile([128, BR, W], BF16, tag="bs")

            # c55: direct 5x5 on n1
            first = True
            for dh in range(5):
                for dw in range(5):
                    in0 = n1[:, 8 + dh:8 + dh + BR, 8 + dw:8 + dw + W]
                    if first:
                        nc.vector.tensor_scalar_mul(
                            bsum[:], in0, col(f"w55_{dh}_{dw}"))
                        first = False
                    else:
                        nc.vector.scalar_tensor_tensor(
                            out=bsum[:], in0=in0, scalar=col(f"w55_{dh}_{dw}"),
                            in1=bsum[:], op0=AO.mult, op1=AO.add)

            # cascaded branches: W-conv into u (with masked inner bias), then
            # H-conv accumulated into bsum
            u = P.tile([128, LR, W], BF16, tag="A")

            def wconv(nrows, h0, ntap, tapf, bias):
                """u[0:nrows] = sum_dw tap[dw]*n1[h0 + r, dw + (P1 - pad) + w] + bias"""
                pad = (ntap - 1) // 2
                # first tap with bias, split by boundary regions
                th = HALO - h0          # top halo rows in u coords
                bh = (SR - HALO) - 64 - h0  # = 75 - h0, bottom halo start
                regions = [
                    (0, 64, 0, th, bias + "t"),
                    (64, 128, 0, th, bias),
                    (0, 128, th, bh, bias),
                    (0, 64, bh, nrows, bias),
                    (64, 128, bh, nrows, bias + "b"),
                ]
                for (p0, p1, r0, r1, bc) in regions:
                    if r0 >= r1:
                        continue
                    nc.vector.tensor_scalar(
                        out=u[p0:p1, r0:r1, :],
                        in0=n1[p0:p1, h0 + r0:h0 + r1, P1 - pad:P1 - pad + W],
                        scalar1=col(tapf(0), p0, p1),
                        scalar2=col(bc, p0, p1),
                        op0=AO.mult, op1=AO.add)
                for dw in range(1, ntap):
                    nc.vector.scalar_tensor_tensor(
                        out=u[:, 0:nrows, :],
                        in0=n1[:, h0:h0 + nrows, P1 - pad + dw:P1 - pad + dw + W],
                        scalar=col(tapf(dw)),
                        in1=u[:, 0:nrows, :],
                        op0=AO.mult, op1=AO.add)

            def hconv(ntap, tapf):
                for dh in range(ntap):
                    nc.vector.scalar_tensor_tensor(
                        out=bsum[:], in0=u[:, dh:dh + BR, :],
                        scalar=col(tapf(dh)), in1=bsum[:],
                        op0=AO.mult, op1=AO.add)

            wconv(72, 7, 7, lambda d: f"w17a_{d}", "b17a")
            hconv(7, lambda d: f"w17b_{d}")
            wconv(76, 5, 11, lambda d: f"w111a_{d}", "b111a")
            hconv(11, lambda d: f"w111b_{d}")
            wconv(86, 0, 21, lambda d: f"w211a_{d}", "b211a")
            hconv(21, lambda d: f"w211b_{d}")

            # ---- mixer (w11) + gating + layer-scale skip -> x_after ----
            xsk = P.tile([128, BR, W], F32, tag="A")
            nc.sync.dma_start(out=xsk[:], in_=x_d[:, RB - 1:RB - 1 + BR, :])
            for k in range(BR // 2):
                ps = PS.tile([128, 2, W], F32, tag="mm")
                nc.tensor.matmul(ps[:], blk(TB_W11 // 128),
                                 bsum[:, 2 * k:2 * k + 2, :],
                                 start=True, stop=True)
                # attn = (mix + b11') * n1   (in place in psum)
                nc.vector.scalar_tensor_tensor(
                    out=ps[:], in0=ps[:], scalar=col("b11p"),
                    in1=n1[:, RB - 1 + 2 * k:RB + 1 + 2 * k, P1:P1 + W],
                    op0=AO.add, op1=AO.mult)
                # x_after = attn * ls1 + x
                nc.vector.scalar_tensor_tensor(
                    out=xsk[:, 2 * k:2 * k + 2, :], in0=ps[:],
                    scalar=col("ls1"), in1=xsk[:, 2 * k:2 * k + 2, :],
                    op0=AO.mult, op1=AO.add)

            # ---- n2 = BN2(x_after), boundary-masked ----
            n2 = P.tile([128, BR, NW], BF16, tag="n1")
            nc.gpsimd.memset(n2[:], 0.0)
            bn2_regions = [
                (0, 64, 0, 1, "t2top"),
                (0, 64, 1, BR, "t2"),
                (64, 128, 0, BR - 1, "t2"),
                (64, 128, BR - 1, BR, "t2bot"),
            ]
            for (p0, p1, r0, r1, bc) in bn2_regions:
                nc.scalar.activation(
                    out=n2[p0:p1, r0:r1, 1:1 + W],
                    in_=xsk[p0:p1, r0:r1, :],
                    func=AF.Identity,
                    bias=col(bc, p0, p1), scale=col("s2", p0, p1))

            # ---- FFN: fw1 (3x3-folded) -> gelu -> fw2 -> skip ----
            t3 = P.tile([128, 4, CH, W], BF16, tag="t3")
            for cc in range(NCH):
                for k in range(CH // 2):
                    row0 = cc * CH + 2 * k      # t3/out row (out-relative)
                    for ti in range(4):
                        psf = PS.tile([128, 2, W], F32, tag="f1")
                        for d in range(9):
                            dh, dw = d // 3, d % 3
                            nc.tensor.matmul(
                                psf[:], blk(ti * 9 + d),
                                n2[:, row0 + dh:row0 + dh + 2, dw:dw + W],
                                start=(d == 0), stop=(d == 8))
                        # gelu(psum + fb1') -> t3, with edge-row bias fixes
                        dst = t3[:, ti, 2 * k:2 * k + 2, :]
                        calls = []
                        if cc == 0 and k == 0:
                            calls = [(0, 64, 0, 1, f"fb1e{ti}"),
                                     (64, 128, 0, 1, f"fb1p{ti}"),
                                     (0, 128, 1, 2, f"fb1p{ti}")]
                        elif cc == NCH - 1 and k == CH // 2 - 1:
                            calls = [(0, 128, 0, 1, f"fb1p{ti}"),
                                     (0, 64, 1, 2, f"fb1p{ti}"),
                                     (64, 128, 1, 2, f"fb1f{ti}")]
                        else:
                            calls = [(0, 128, 0, 2, f"fb1p{ti}")]
                        for (p0, p1, r0, r1, bc) in calls:
                            nc.scalar.activation(
                                out=dst[p0:p1, r0:r1, :],
                                in_=psf[p0:p1, r0:r1, :],
                                func=AF.Gelu, bias=col(bc, p0, p1), scale=1.0)
                ost = P.tile([128, CH, W], F32, tag="bs")
                for k in range(CH // 2):
                    pso = PS.tile([128, 2, W], F32, tag="f2")
                    for ti in range(4):
                        nc.tensor.matmul(
                            pso[:], blk(TB_FW2 // 128 + ti),
                            t3[:, ti, 2 * k:2 * k + 2, :],
                            start=(ti == 0), stop=(ti == 3))
                    # y = psum * ls2 + ls2*fb2
                    nc.scalar.activation(
                        out=ost[:, 2 * k:2 * k + 2, :], in_=pso[:],
                        func=AF.Identity, bias=col("fb2p"), scale=col("ls2"))
                # += x_after rows
                nc.vector.tensor_add(
                    ost[:], ost[:], xsk[:, cc * CH + 1:cc * CH + 1 + CH, :])
                nc.sync.dma_start(out=o_d[:, cc * CH:cc * CH + CH, :],
                                  in_=ost[:])
    nc.compile()
    return nc


_NC_CACHE = None


def _get_nc():
    global _NC_CACHE
    if _NC_CACHE is None:
        _NC_CACHE = build_nc()
    return _NC_CACHE


# ---------------- host side ----------------
def _prep_core(inputs, b, half, params):
    """Per-core input dict."""
    x = inputs["x"]
    # x slice with halo rows, zero padded at image edges, arranged (s, c)
    r0 = 128 * half - HALO
    xs = np.zeros((2, C, LR, W), np.float32)
    for s in range(2):
        lo, hi = r0 + 64 * s, r0 + 64 * s + LR
        clo, chi = max(lo, 0), min(hi, 256)
        if clo < chi:
            xs[s, :, clo - lo:chi - lo, :] = x[b, :, clo:chi, :]
    cvec = params["cvec_top"] if half == 0 else params["cvec_bot"]
    return {"xs": xs.reshape(128, LR, W),
            "cvec": cvec, "tabs": params["tabs"]}


def _prep_params(inputs):
    ii = {k: np.asarray(v, np.float64) for k, v in inputs.items()}
    s1 = ii["g1"] / np.sqrt(ii["v1"] + EPS)
    t1 = ii["b1"] - ii["m1"] * s1
    s2 = ii["g2"] / np.sqrt(ii["v2"] + EPS)
    t2 = ii["b2"] - ii["m2"] * s2
    w55 = ii["w55"][:, 0]          # (C, 5, 5)
    w17a = ii["w17a"][:, 0, 0]     # (C, 7)
    w17b = ii["w17b"][:, 0, :, 0]  # (C, 7)
    w111a = ii["w111a"][:, 0, 0]
    w111b = ii["w111b"][:, 0, :, 0]
    w211a = ii["w211a"][:, 0, 0]
    w211b = ii["w211b"][:, 0, :, 0]
    w3 = ii["fdw"][:, 0]           # (HID, 3, 3)
    b0 = ii["bb55"] + ii["b17b"] + ii["b111b"] + ii["b211b"]
    b11p = ii["b11"] + ii["w11"] @ b0
    sall = w3.sum(axis=(1, 2))
    s_notop = w3[:, 1:, :].sum(axis=(1, 2))
    s_nobot = w3[:, :2, :].sum(axis=(1, 2))
    fb1p = ii["fbdw"] + ii["fb1"] * sall
    fb1e = ii["fbdw"] + ii["fb1"] * s_notop
    fb1f = ii["fbdw"] + ii["fb1"] * s_nobot

    def dup(v):
        return np.concatenate([v, v]).astype(np.float32)

    def cvec_for(half):
        cvb = np.zeros((128, NCOL), np.float32)

        def setc(name, v):
            cvb[:, _COLS[name]] = v

        top, bot = (half == 0), (half == 1)
        setc("s1", dup(s1)); setc("t1", dup(t1))
        setc("t1top", dup(t1 * (0.0 if top else 1.0)))
        setc("t1bot", dup(t1 * (0.0 if bot else 1.0)))
        setc("s2", dup(s2)); setc("t2", dup(t2))
        setc("t2top", dup(t2 * (0.0 if top else 1.0)))
        setc("t2bot", dup(t2 * (0.0 if bot else 1.0)))
        setc("b11p", dup(b11p)); setc("ls1", dup(ii["ls1"]))
        setc("ls2", dup(ii["ls2"])); setc("fb2p", dup(ii["ls2"] * ii["fb2"]))
        for nm, bb in (("b17a", ii["b17a"]), ("b111a", ii["b111a"]),
                       ("b211a", ii["b211a"])):
            setc(nm, dup(bb))
            setc(nm + "t", dup(bb * (0.0 if top else 1.0)))
            setc(nm + "b", dup(bb * (0.0 if bot else 1.0)))
        for t in range(4):
            j = slice(64 * t, 64 * t + 64)
            setc(f"fb1p{t}", dup(fb1p[j]))
            setc(f"fb1e{t}", dup(fb1e[j] if top else fb1p[j]))
            setc(f"fb1f{t}", dup(fb1f[j] if bot else fb1p[j]))
        for dh in range(5):
            for dw in range(5):
                setc(f"w55_{dh}_{dw}", dup(w55[:, dh, dw]))
        for d in range(7):
            setc(f"w17a_{d}", dup(w17a[:, d]))
            setc(f"w17b_{d}", dup(w17b[:, d]))
        for d in range(11):
            setc(f"w111a_{d}", dup(w111a[:, d]))
            setc(f"w111b_{d}", dup(w111b[:, d]))
        for d in range(21):
            setc(f"w211a_{d}", dup(w211a[:, d]))
            setc(f"w211b_{d}", dup(w211b[:, d]))
        return cvb

    # tabs: block-diagonal (over s) lhsT weight tables, bf16
    tabs = np.zeros((128, TBN), np.float64)
    fw1 = ii["fw1"]   # (HID, C)
    fw2 = ii["fw2"]   # (C, HID)
    w11 = ii["w11"]   # (C, C)

    def bd(m):  # (K, M) -> block diag over s: [(s,K), (s,M)]
        z = np.zeros((2 * m.shape[0], 2 * m.shape[1]))
        z[:m.shape[0], :m.shape[1]] = m
        z[m.shape[0]:, m.shape[1]:] = m
        return z

    for ti in range(4):
        j = slice(64 * ti, 64 * ti + 64)
        for d in range(9):
            dh, dw = d // 3, d % 3
            # lhsT[(s,c), (s,j)] = fw1[j, c] * w3[j, dh, dw]
            m = (fw1[j, :] * w3[j, dh, dw][:, None]).T   # (C, 64)
            tabs[:, (TB_FW1 + (ti * 9 + d) * 128):][:, :128] = bd(m)
        m2 = fw2[:, j]                                    # (C, 64) -> (K=j, M=c)
        tabs[:, (TB_FW2 + ti * 128):][:, :128] = bd(m2.T)
    tabs[:, TB_W11:TB_W11 + 128] = bd(w11.T)

    return {"cvec_top": cvec_for(0), "cvec_bot": cvec_for(1),
            "tabs": tabs.astype(BF)}


LAST_RESULTS = None


def _ensure_ntff_hook():
    """Recreate the missing antenv.axon_hooks registry and install the
    ctypes NTFF profiling hook (profiling-only; not needed to run)."""
    import sys
    import types
    try:
        from antenv.axon_hooks import get_axon_ntff_profile_hook  # noqa: F401
        return
    except ImportError:
        pass
    import antenv
    mod = types.ModuleType("antenv.axon_hooks")
    _hook_box = [None]
    mod.set_axon_ntff_profile_hook = lambda h: _hook_box.__setitem__(0, h)
    mod.get_axon_ntff_profile_hook = lambda: _hook_box[0]
    sys.modules["antenv.axon_hooks"] = mod
    antenv.axon_hooks = mod
    sys.path.insert(0, "/root/.axon_site/trn_agent_boot")
    try:
        import trn_boot
        hook = trn_boot._ntff_profile_via_ctypes("/opt/axon/libaxon_pjrt.so")
        mod.set_axon_ntff_profile_hook(hook)
    except Exception as e:  # pragma: no cover
        print("ntff hook install failed:", e)


def kernel(**inputs) -> np.ndarray:
    global LAST_RESULTS
    nc = _get_nc()
    params = _prep_params(inputs)
    in_maps = []
    for core in range(8):
        b, half = core // 2, core % 2
        in_maps.append(_prep_core(inputs, b, half, params))
    import os
    trace = bool(int(os.environ.get("KTRACE", "0")))
    if trace:
        _ensure_ntff_hook()
    res = run_bass_kernel_spmd(nc, in_maps, core_ids=list(range(8)),
                               trace=trace)
    LAST_RESULTS = res
    out = np.zeros((4, C, 256, W), np.float32)
    for core in range(8):
        b, half = core // 2, core % 2
        o = res.results[core]["out"].reshape(2, C, 64, W)
        for s in range(2):
            r = 128 * half + 64 * s
            out[b, :, r:r + 64, :] = o[s]
    return out


# revision 11
# speedup vs baseline: 1.4493x; 1.4493x over previous
"""Bass/Trainium2 kernel for nn_Block_60224031424641 (SegNeXt MSCAN block).

Reference computation (per image, NCHW, C=64, H=W=256):
  n1   = BN(x)                                (eval-mode batchnorm)
  c55  = dw5x5(n1) + bb55
  c17  = dw7x1(dw1x7(n1) + b17a) + b17b       (and 11, 21 analogues)
  mix  = 1x1(c55 + c17 + c111 + c211) + b11
  x    = x + ls1 * (mix * n1)
  n2   = BN2(x)
  hdn  = gelu(dw3x3(1x1(n2) + fb1) + fbdw)
  out  = x + ls2 * (1x1(hdn) + fb2)

Sharding: 8 cores = (batch 4) x (image h-half 2), pure data parallel with
host-provided halo rows (no cross-core communication).

Per-core layout: SBUF partitions = (s, c) where s in {0,1} is a further
h-quarter split and c the 64 channels; free dim = (rows, w).  All conv
shifts are free-dim offsets; per-channel conv taps are per-partition
scalars consumed by DVE scalar_tensor_tensor ops.  1x1 convs run on the
tensor engine with block-diagonal (over s) lhsT weights; the FFN's 3x3
depthwise conv is folded into the fw1 matmul (9 accumulating matmuls
with shifted rhs).  Image-boundary zero-padding is handled by per-core
bias columns (out-of-image regions get a zeroed bias so BN/bias never
re-introduces nonzeros where the reference zero-pads).
"""

import numpy as np
import ml_dtypes

import concourse.bass as bass
import concourse.bacc as bacc
import concourse.mybir as mybir
import concourse.tile as tile
from concourse.bass_utils import run_bass_kernel_spmd

F32 = mybir.dt.float32
BF16 = mybir.dt.bfloat16
AO = mybir.AluOpType
AF = mybir.ActivationFunctionType
BF = ml_dtypes.bfloat16

# ---------------- geometry ----------------
C = 64          # channels
W = 256         # image width
HALO = 11       # input halo rows each side (10 conv + 1 ffn)
SR = 128 + 2 * HALO          # 150 slice rows per core
LR = 64 + 2 * HALO           # 86 rows per (s) half
WP = 276        # n1 padded width (10 each side)
P1 = 10         # n1 left pad
BR = 66         # bsum / attn / n2 rows (out-relative [-1, 65))
NW = 258        # n2 padded width (1 each side)
RB = HALO       # local row of first out row (11)
CH = 4          # ffn chunk rows
NCH = 16        # ffn chunks (4*16 = 64 out rows per half)
EPS = 1e-5

# ---------------- cvec column registry ----------------
_COLS: dict[str, int] = {}


def _col(name: str) -> int:
    if name not in _COLS:
        _COLS[name] = len(_COLS)
    return _COLS[name]


def _build_cols():
    for n in ("s1", "t1", "t1top", "t1bot", "s2", "t2", "t2top", "t2bot",
              "b11p", "ls1", "ls2", "fb2p",
              "b17a", "b17at", "b17ab",
              "b111a", "b111at", "b111ab",
              "b211a", "b211at", "b211ab"):
        _col(n)
    for t in range(4):
        _col(f"fb1p{t}")
        _col(f"fb1e{t}")
        _col(f"fb1f{t}")
    for dh in range(5):
        for dw in range(5):
            _col(f"w55_{dh}_{dw}")
    for dw in range(7):
        _col(f"w17a_{dw}")
    for dh in range(7):
        _col(f"w17b_{dh}")
    for dw in range(11):
        _col(f"w111a_{dw}")
    for dh in range(11):
        _col(f"w111b_{dh}")
    for dw in range(21):
        _col(f"w211a_{dw}")
    for dh in range(21):
        _col(f"w211b_{dh}")


_build_cols()
NCOL = len(_COLS)

# tabs (bf16 matmul weight tables):
#   64 mixer-fold blocks (w11 composed with per-channel conv taps):
#     25 c55 (dh*5+dw), 7 c17b, 11 c111b, 21 c211b
#   36 fw1-fold blocks, 4 fw2 blocks
TB_FOLD = 0
TB_F17 = 25
TB_F111 = 32
TB_F211 = 43
TB_FW1 = 64               # + (ti*9 + d), d = dh*3+dw
TB_FW2 = 100              # + ti
TBN = 104 * 128


# ---------------- device kernel ----------------
def build_nc():
    nc = bacc.Bacc("TRN2")
    x_d = nc.dram_tensor("xs", [128, LR, W], F32, kind="ExternalInput")
    cv_d = nc.dram_tensor("cvec", [128, NCOL], F32, kind="ExternalInput")
    tb_d = nc.dram_tensor("tabs", [128, TBN], BF16, kind="ExternalInput")
    o_d = nc.dram_tensor("out", [128, 64, W], F32, kind="ExternalOutput")

    with tile.TileContext(nc) as tc:
        with tc.tile_pool(name="P", bufs=1) as P, \
             tc.tile_pool(name="XST", bufs=1) as XST, \
             tc.tile_pool(name="PS", bufs=2, space="PSUM") as PS:

            cv = P.tile([128, NCOL], F32, tag="cv")
            nc.sync.dma_start(out=cv[:], in_=cv_d[:])
            tb = P.tile([128, TBN], BF16, tag="tb")
            nc.sync.dma_start(out=tb[:], in_=tb_d[:])

            def col(name, p0=0, p1=128):
                i = _COLS[name]
                return cv[p0:p1, i:i + 1]

            def blk(i):
                return tb[:, i * 128:(i + 1) * 128]

            # ---- n1 = BN1(x), streamed, with boundary-masked bias ----
            n1 = P.tile([128, LR, WP], BF16, tag="n1")
            nc.gpsimd.memset(n1[:], 0.0)
            # region table: (p0, p1, r0, r1, biascol); rows are local [0, 86)
            bn1_regions = [
                (0, 64, 0, HALO, "t1top"),
                (0, 64, HALO, LR, "t1"),
                (64, 128, 0, LR - HALO, "t1"),
                (64, 128, LR - HALO, LR, "t1bot"),
            ]
            nchunk = (LR + CH - 1) // CH
            for k in range(nchunk):
                r0, r1 = k * CH, min((k + 1) * CH, LR)
                xst = XST.tile([128, CH, W], F32, tag="xst")
                nc.sync.dma_start(out=xst[:, :r1 - r0, :], in_=x_d[:, r0:r1, :])
                for (p0, p1, g0, g1, bc) in bn1_regions:
                    a0, a1 = max(g0, r0), min(g1, r1)
                    if a0 >= a1:
                        continue
                    nc.scalar.activation(
                        out=n1[p0:p1, a0:a1, P1:P1 + W],
                        in_=xst[p0:p1, a0 - r0:a1 - r0, :],
                        func=AF.Identity,
                        bias=col(bc, p0, p1),
                        scale=col("s1", p0, p1),
                    )

            # ---- depthwise conv stack ----
            # bsum accumulates the MIXER output directly: every branch's
            # H-taps (and all of c55) are folded into PE matmuls whose lhsT
            # is w11 composed with the per-channel tap (block-diag over s).
            bsum = P.tile([128, BR, W], BF16, tag="bs")

            def fold(b0, ntaps, rhs_fn, first):
                for k in range(BR // 2):
                    ps = PS.tile([128, 2, W], F32, tag="mm")
                    for t in range(ntaps):
                        nc.tensor.matmul(ps[:], blk(b0 + t), rhs_fn(t, k),
                                         start=(t == 0), stop=(t == ntaps - 1))
                    dst = bsum[:, 2 * k:2 * k + 2, :]
                    if first:
                        nc.vector.tensor_copy(dst, ps[:])
                    else:
                        nc.vector.tensor_add(dst, dst, ps[:])

            # c55 (5x5 on n1) fully folded
            fold(TB_FOLD, 25,
                 lambda t, k: n1[:, 8 + t // 5 + 2 * k:10 + t // 5 + 2 * k,
                                 8 + t % 5:8 + t % 5 + W],
                 True)

            # cascaded branches: W-conv into u (with masked inner bias) on
            # DVE, then H-conv x w11 folded on PE
            u = P.tile([128, LR, W], BF16, tag="A")

            def wconv(nrows, h0, ntap, tapf, bias):
                """u[0:nrows] = sum_dw tap[dw]*n1[h0 + r, dw + (P1 - pad) + w] + bias"""
                pad = (ntap - 1) // 2
                # first tap with bias, split by boundary regions
                th = HALO - h0          # top halo rows in u coords
                bh = (SR - HALO) - 64 - h0  # = 75 - h0, bottom halo start
                regions = [
                    (0, 64, 0, th, bias + "t"),
                    (64, 128, 0, th, bias),
                    (0, 128, th, bh, bias),
                    (0, 64, bh, nrows, bias),
                    (64, 128, bh, nrows, bias + "b"),
                ]
                for (p0, p1, r0, r1, bc) in regions:
                    if r0 >= r1:
                        continue
                    nc.vector.tensor_scalar(
                        out=u[p0:p1, r0:r1, :],
                        in0=n1[p0:p1, h0 + r0:h0 + r1, P1 - pad:P1 - pad + W],
                        scalar1=col(tapf(0), p0, p1),
                        scalar2=col(bc, p0, p1),
                        op0=AO.mult, op1=AO.add)
                for dw in range(1, ntap):
                    nc.vector.scalar_tensor_tensor(
                        out=u[:, 0:nrows, :],
                        in0=n1[:, h0:h0 + nrows, P1 - pad + dw:P1 - pad + dw + W],
                        scalar=col(tapf(dw)),
                        in1=u[:, 0:nrows, :],
                        op0=AO.mult, op1=AO.add)

            wconv(72, 7, 7, lambda d: f"w17a_{d}", "b17a")
            fold(TB_F17, 7, lambda t, k: u[:, t + 2 * k:t + 2 * k + 2, :],
                 False)
            wconv(76, 5, 11, lambda d: f"w111a_{d}", "b111a")
            fold(TB_F111, 11, lambda t, k: u[:, t + 2 * k:t + 2 * k + 2, :],
                 False)
            wconv(86, 0, 21, lambda d: f"w211a_{d}", "b211a")
            fold(TB_F211, 21, lambda t, k: u[:, t + 2 * k:t + 2 * k + 2, :],
                 False)

            # ---- gating + layer-scale skip -> x_after ----
            xsk = P.tile([128, BR, W], F32, tag="A")
            nc.sync.dma_start(out=xsk[:], in_=x_d[:, RB - 1:RB - 1 + BR, :])
            for k in range(BR // 2):
                ps = PS.tile([128, 2, W], F32, tag="g")
                # attn = (mix + b11') * n1   (psum as scratch)
                nc.vector.scalar_tensor_tensor(
                    out=ps[:], in0=bsum[:, 2 * k:2 * k + 2, :],
                    scalar=col("b11p"),
                    in1=n1[:, RB - 1 + 2 * k:RB + 1 + 2 * k, P1:P1 + W],
                    op0=AO.add, op1=AO.mult)
                # x_after = attn * ls1 + x
                nc.vector.scalar_tensor_tensor(
                    out=xsk[:, 2 * k:2 * k + 2, :], in0=ps[:],
                    scalar=col("ls1"), in1=xsk[:, 2 * k:2 * k + 2, :],
                    op0=AO.mult, op1=AO.add)

            # ---- n2 = BN2(x_after), boundary-masked ----
            n2 = P.tile([128, BR, NW], BF16, tag="n1")
            nc.gpsimd.memset(n2[:], 0.0)
            bn2_regions = [
                (0, 64, 0, 1, "t2top"),
                (0, 64, 1, BR, "t2"),
                (64, 128, 0, BR - 1, "t2"),
                (64, 128, BR - 1, BR, "t2bot"),
            ]
            for (p0, p1, r0, r1, bc) in bn2_regions:
                nc.scalar.activation(
                    out=n2[p0:p1, r0:r1, 1:1 + W],
                    in_=xsk[p0:p1, r0:r1, :],
                    func=AF.Identity,
                    bias=col(bc, p0, p1), scale=col("s2", p0, p1))

            # ---- FFN: fw1 (3x3-folded) -> gelu -> fw2 -> skip ----
            t3 = P.tile([128, 4, CH, W], BF16, tag="t3")
            for cc in range(NCH):
                for k in range(CH // 2):
                    row0 = cc * CH + 2 * k      # t3/out row (out-relative)
                    for ti in range(4):
                        psf = PS.tile([128, 2, W], F32, tag="f1")
                        for d in range(9):
                            dh, dw = d // 3, d % 3
                            nc.tensor.matmul(
                                psf[:], blk(ti * 9 + d),
                                n2[:, row0 + dh:row0 + dh + 2, dw:dw + W],
                                start=(d == 0), stop=(d == 8))
                        # gelu(psum + fb1') -> t3, with edge-row bias fixes
                        dst = t3[:, ti, 2 * k:2 * k + 2, :]
                        calls = []
                        if cc == 0 and k == 0:
                            calls = [(0, 64, 0, 1, f"fb1e{ti}"),
                                     (64, 128, 0, 1, f"fb1p{ti}"),
                                     (0, 128, 1, 2, f"fb1p{ti}")]
                        elif cc == NCH - 1 and k == CH // 2 - 1:
                            calls = [(0, 128, 0, 1, f"fb1p{ti}"),
                                     (0, 64, 1, 2, f"fb1p{ti}"),
                                     (64, 128, 1, 2, f"fb1f{ti}")]
                        else:
                            calls = [(0, 128, 0, 2, f"fb1p{ti}")]
                        for (p0, p1, r0, r1, bc) in calls:
                            nc.scalar.activation(
                                out=dst[p0:p1, r0:r1, :],
                                in_=psf[p0:p1, r0:r1, :],
                                func=AF.Gelu, bias=col(bc, p0, p1), scale=1.0)
                ost = P.tile([128, CH, W], F32, tag="bs")
                for k in range(CH // 2):
                    pso = PS.tile([128, 2, W], F32, tag="f2")
                    for ti in range(4):
                        nc.tensor.matmul(
                            pso[:], blk(TB_FW2 // 128 + ti),
                            t3[:, ti, 2 * k:2 * k + 2, :],
                            start=(ti == 0), stop=(ti == 3))
                    # y = psum * ls2 + ls2*fb2
                    nc.scalar.activation(
                        out=ost[:, 2 * k:2 * k + 2, :], in_=pso[:],
                        func=AF.Identity, bias=col("fb2p"), scale=col("ls2"))
                # += x_after rows
                nc.vector.tensor_add(
                    ost[:], ost[:], xsk[:, cc * CH + 1:cc * CH + 1 + CH, :])
                nc.sync.dma_start(out=o_d[:, cc * CH:cc * CH + CH, :],
                                  in_=ost[:])
    nc.compile()
    return nc


_NC_CACHE = None


def _get_nc():
    global _NC_CACHE
    if _NC_CACHE is None:
        _NC_CACHE = build_nc()
    return _NC_CACHE


# ---------------- host side ----------------
def _prep_core(inputs, b, half, params):
    """Per-core input dict."""
    x = inputs["x"]
    # x slice with halo rows, zero padded at image edges, arranged (s, c)
    r0 = 128 * half - HALO
    xs = np.zeros((2, C, LR, W), np.float32)
    for s in range(2):
        lo, hi = r0 + 64 * s, r0 + 64 * s + LR
        clo, chi = max(lo, 0), min(hi, 256)
        if clo < chi:
            xs[s, :, clo - lo:chi - lo, :] = x[b, :, clo:chi, :]
    cvec = params["cvec_top"] if half == 0 else params["cvec_bot"]
    return {"xs": xs.reshape(128, LR, W),
            "cvec": cvec, "tabs": params["tabs"]}


def _prep_params(inputs):
    ii = {k: np.asarray(v, np.float64) for k, v in inputs.items()}
    s1 = ii["g1"] / np.sqrt(ii["v1"] + EPS)
    t1 = ii["b1"] - ii["m1"] * s1
    s2 = ii["g2"] / np.sqrt(ii["v2"] + EPS)
    t2 = ii["b2"] - ii["m2"] * s2
    w55 = ii["w55"][:, 0]          # (C, 5, 5)
    w17a = ii["w17a"][:, 0, 0]     # (C, 7)
    w17b = ii["w17b"][:, 0, :, 0]  # (C, 7)
    w111a = ii["w111a"][:, 0, 0]
    w111b = ii["w111b"][:, 0, :, 0]
    w211a = ii["w211a"][:, 0, 0]
    w211b = ii["w211b"][:, 0, :, 0]
    w3 = ii["fdw"][:, 0]           # (HID, 3, 3)
    b0 = ii["bb55"] + ii["b17b"] + ii["b111b"] + ii["b211b"]
    b11p = ii["b11"] + ii["w11"] @ b0
    sall = w3.sum(axis=(1, 2))
    s_notop = w3[:, 1:, :].sum(axis=(1, 2))
    s_nobot = w3[:, :2, :].sum(axis=(1, 2))
    fb1p = ii["fbdw"] + ii["fb1"] * sall
    fb1e = ii["fbdw"] + ii["fb1"] * s_notop
    fb1f = ii["fbdw"] + ii["fb1"] * s_nobot

    def dup(v):
        return np.concatenate([v, v]).astype(np.float32)

    def cvec_for(half):
        cvb = np.zeros((128, NCOL), np.float32)

        def setc(name, v):
            cvb[:, _COLS[name]] = v

        top, bot = (half == 0), (half == 1)
        setc("s1", dup(s1)); setc("t1", dup(t1))
        setc("t1top", dup(t1 * (0.0 if top else 1.0)))
        setc("t1bot", dup(t1 * (0.0 if bot else 1.0)))
        setc("s2", dup(s2)); setc("t2", dup(t2))
        setc("t2top", dup(t2 * (0.0 if top else 1.0)))
        setc("t2bot", dup(t2 * (0.0 if bot else 1.0)))
        setc("b11p", dup(b11p)); setc("ls1", dup(ii["ls1"]))
        setc("ls2", dup(ii["ls2"])); setc("fb2p", dup(ii["ls2"] * ii["fb2"]))
        for nm, bb in (("b17a", ii["b17a"]), ("b111a", ii["b111a"]),
                       ("b211a", ii["b211a"])):
            setc(nm, dup(bb))
            setc(nm + "t", dup(bb * (0.0 if top else 1.0)))
            setc(nm + "b", dup(bb * (0.0 if bot else 1.0)))
        for t in range(4):
            j = slice(64 * t, 64 * t + 64)
            setc(f"fb1p{t}", dup(fb1p[j]))
            setc(f"fb1e{t}", dup(fb1e[j] if top else fb1p[j]))
            setc(f"fb1f{t}", dup(fb1f[j] if bot else fb1p[j]))
        for dh in range(5):
            for dw in range(5):
                setc(f"w55_{dh}_{dw}", dup(w55[:, dh, dw]))
        for d in range(7):
            setc(f"w17a_{d}", dup(w17a[:, d]))
            setc(f"w17b_{d}", dup(w17b[:, d]))
        for d in range(11):
            setc(f"w111a_{d}", dup(w111a[:, d]))
            setc(f"w111b_{d}", dup(w111b[:, d]))
        for d in range(21):
            setc(f"w211a_{d}", dup(w211a[:, d]))
            setc(f"w211b_{d}", dup(w211b[:, d]))
        return cvb

    # tabs: block-diagonal (over s) lhsT weight tables, bf16
    tabs = np.zeros((128, TBN), np.float64)
    fw1 = ii["fw1"]   # (HID, C)
    fw2 = ii["fw2"]   # (C, HID)
    w11 = ii["w11"]   # (C, C)

    def bd(m):  # (K, M) -> block diag over s: [(s,K), (s,M)]
        z = np.zeros((2 * m.shape[0], 2 * m.shape[1]))
        z[:m.shape[0], :m.shape[1]] = m
        z[m.shape[0]:, m.shape[1]:] = m
        return z

    # mixer-fold tables: lhsT[(s,c), (s,o)] = w11[o, c] * tap[c]
    w11T = w11.T                                          # (c, o)

    def setblk(i, m):
        tabs[:, i * 128:(i + 1) * 128] = bd(m)

    for dh in range(5):
        for dw in range(5):
            setblk(TB_FOLD + dh * 5 + dw, w11T * w55[:, dh, dw][:, None])
    for dh in range(7):
        setblk(TB_F17 + dh, w11T * w17b[:, dh][:, None])
    for dh in range(11):
        setblk(TB_F111 + dh, w11T * w111b[:, dh][:, None])
    for dh in range(21):
        setblk(TB_F211 + dh, w11T * w211b[:, dh][:, None])

    for ti in range(4):
        j = slice(64 * ti, 64 * ti + 64)
        for d in range(9):
            dh, dw = d // 3, d % 3
            # lhsT[(s,c), (s,j)] = fw1[j, c] * w3[j, dh, dw]
            m = (fw1[j, :] * w3[j, dh, dw][:, None]).T   # (C, 64)
            setblk(TB_FW1 + ti * 9 + d, m)
        m2 = fw2[:, j]                                    # (C, 64) -> (K=j, M=c)
        setblk(TB_FW2 + ti, m2.T)

    return {"cvec_top": cvec_for(0), "cvec_bot": cvec_for(1),
            "tabs": tabs.astype(BF)}


LAST_RESULTS = None


def _ensure_ntff_hook():
    """Recreate the missing antenv.axon_hooks registry and install the
    ctypes NTFF profiling hook (profiling-only; not needed to run)."""
    import sys
    import types
    try:
        from antenv.axon_hooks import get_axon_ntff_profile_hook  # noqa: F401
        return
    except ImportError:
        pass
    import antenv
    mod = types.ModuleType("antenv.axon_hooks")
    _hook_box = [None]
    mod.set_axon_ntff_profile_hook = lambda h: _hook_box.__setitem__(0, h)
    mod.get_axon_ntff_profile_hook = lambda: _hook_box[0]
    sys.modules["antenv.axon_hooks"] = mod
    antenv.axon_hooks = mod
    sys.path.insert(0, "/root/.axon_site/trn_agent_boot")
    try:
        import trn_boot
        hook = trn_boot._ntff_profile_via_ctypes("/opt/axon/libaxon_pjrt.so")
        mod.set_axon_ntff_profile_hook(hook)
    except Exception as e:  # pragma: no cover
        print("ntff hook install failed:", e)


def kernel(**inputs) -> np.ndarray:
    global LAST_RESULTS
    nc = _get_nc()
    params = _prep_params(inputs)
    in_maps = []
    for core in range(8):
        b, half = core // 2, core % 2
        in_maps.append(_prep_core(inputs, b, half, params))
    import os
    trace = bool(int(os.environ.get("KTRACE", "0")))
    if trace:
        _ensure_ntff_hook()
    res = run_bass_kernel_spmd(nc, in_maps, core_ids=list(range(8)),
                               trace=trace)
    LAST_RESULTS = res
    out = np.zeros((4, C, 256, W), np.float32)
    for core in range(8):
        b, half = core // 2, core % 2
        o = res.results[core]["out"].reshape(2, C, 64, W)
        for s in range(2):
            r = 128 * half + 64 * s
            out[b, :, r:r + 64, :] = o[s]
    return out


# revision 13
# speedup vs baseline: 1.4572x; 1.0054x over previous
"""Bass/Trainium2 kernel for nn_Block_60224031424641 (SegNeXt MSCAN block).

Reference computation (per image, NCHW, C=64, H=W=256):
  n1   = BN(x)                                (eval-mode batchnorm)
  c55  = dw5x5(n1) + bb55
  c17  = dw7x1(dw1x7(n1) + b17a) + b17b       (and 11, 21 analogues)
  mix  = 1x1(c55 + c17 + c111 + c211) + b11
  x    = x + ls1 * (mix * n1)
  n2   = BN2(x)
  hdn  = gelu(dw3x3(1x1(n2) + fb1) + fbdw)
  out  = x + ls2 * (1x1(hdn) + fb2)

Sharding: 8 cores = (batch 4) x (image h-half 2), pure data parallel with
host-provided halo rows (no cross-core communication).

Per-core layout: SBUF partitions = (s, c) where s in {0,1} is a further
h-quarter split and c the 64 channels; free dim = (rows, w).  All conv
shifts are free-dim offsets; per-channel conv taps are per-partition
scalars consumed by DVE scalar_tensor_tensor ops.  1x1 convs run on the
tensor engine with block-diagonal (over s) lhsT weights; the FFN's 3x3
depthwise conv is folded into the fw1 matmul (9 accumulating matmuls
with shifted rhs).  Image-boundary zero-padding is handled by per-core
bias columns (out-of-image regions get a zeroed bias so BN/bias never
re-introduces nonzeros where the reference zero-pads).
"""

import numpy as np
import ml_dtypes

import concourse.bass as bass
import concourse.bacc as bacc
import concourse.mybir as mybir
import concourse.tile as tile
from concourse.bass_utils import run_bass_kernel_spmd

F32 = mybir.dt.float32
BF16 = mybir.dt.bfloat16
AO = mybir.AluOpType
AF = mybir.ActivationFunctionType
BF = ml_dtypes.bfloat16

# ---------------- geometry ----------------
C = 64          # channels
W = 256         # image width
HALO = 11       # input halo rows each side (10 conv + 1 ffn)
SR = 128 + 2 * HALO          # 150 slice rows per core
LR = 64 + 2 * HALO           # 86 rows per (s) half
WP = 276        # n1 padded width (10 each side)
P1 = 10         # n1 left pad
BR = 66         # bsum / attn / n2 rows (out-relative [-1, 65))
NW = 258        # n2 padded width (1 each side)
RB = HALO       # local row of first out row (11)
CH = 4          # ffn chunk rows
NCH = 16        # ffn chunks (4*16 = 64 out rows per half)
EPS = 1e-5

# ---------------- cvec column registry ----------------
_COLS: dict[str, int] = {}


def _col(name: str) -> int:
    if name not in _COLS:
        _COLS[name] = len(_COLS)
    return _COLS[name]


def _build_cols():
    for n in ("s1", "t1", "t1top", "t1bot", "s2", "t2", "t2top", "t2bot",
              "b11p", "ls1", "ls2", "fb2p",
              "b17a", "b17at", "b17ab",
              "b111a", "b111at", "b111ab",
              "b211a", "b211at", "b211ab"):
        _col(n)
    for t in range(4):
        _col(f"fb1p{t}")
        _col(f"fb1e{t}")
        _col(f"fb1f{t}")
    for dh in range(5):
        for dw in range(5):
            _col(f"w55_{dh}_{dw}")
    for dw in range(7):
        _col(f"w17a_{dw}")
    for dh in range(7):
        _col(f"w17b_{dh}")
    for dw in range(11):
        _col(f"w111a_{dw}")
    for dh in range(11):
        _col(f"w111b_{dh}")
    for dw in range(21):
        _col(f"w211a_{dw}")
    for dh in range(21):
        _col(f"w211b_{dh}")


_build_cols()
NCOL = len(_COLS)

# tabs (bf16 matmul weight tables):
#   64 mixer-fold blocks (w11 composed with per-channel conv taps):
#     25 c55 (dh*5+dw), 7 c17b, 11 c111b, 21 c211b
#   36 fw1-fold blocks, 4 fw2 blocks
TB_FOLD = 0
TB_F17 = 25
TB_F111 = 32
TB_F211 = 43
TB_FW1 = 64               # + (ti*9 + d), d = dh*3+dw
TB_FW2 = 100              # + ti
TBN = 104 * 128


# ---------------- device kernel ----------------
def build_nc():
    nc = bacc.Bacc("TRN2")
    x_d = nc.dram_tensor("xs", [128, LR, W], F32, kind="ExternalInput")
    cv_d = nc.dram_tensor("cvec", [128, NCOL], F32, kind="ExternalInput")
    tb_d = nc.dram_tensor("tabs", [128, TBN], BF16, kind="ExternalInput")
    o_d = nc.dram_tensor("out", [128, 64, W], F32, kind="ExternalOutput")

    with tile.TileContext(nc) as tc:
        with tc.tile_pool(name="P", bufs=1) as P, \
             tc.tile_pool(name="XST", bufs=1) as XST, \
             tc.tile_pool(name="PS", bufs=6, space="PSUM") as PS:

            cv = P.tile([128, NCOL], F32, tag="cv")
            nc.sync.dma_start(out=cv[:], in_=cv_d[:])
            tb = P.tile([128, TBN], BF16, tag="tb")
            nc.sync.dma_start(out=tb[:], in_=tb_d[:])

            def col(name, p0=0, p1=128):
                i = _COLS[name]
                return cv[p0:p1, i:i + 1]

            def blk(i):
                return tb[:, i * 128:(i + 1) * 128]

            # ---- n1 = BN1(x), streamed, with boundary-masked bias ----
            n1 = P.tile([128, LR, WP], BF16, tag="n1")
            nc.gpsimd.memset(n1[:], 0.0)
            # region table: (p0, p1, r0, r1, biascol); rows are local [0, 86)
            bn1_regions = [
                (0, 64, 0, HALO, "t1top"),
                (0, 64, HALO, LR, "t1"),
                (64, 128, 0, LR - HALO, "t1"),
                (64, 128, LR - HALO, LR, "t1bot"),
            ]
            nchunk = (LR + CH - 1) // CH
            for k in range(nchunk):
                r0, r1 = k * CH, min((k + 1) * CH, LR)
                xst = XST.tile([128, CH, W], F32, tag="xst")
                nc.sync.dma_start(out=xst[:, :r1 - r0, :], in_=x_d[:, r0:r1, :])
                for (p0, p1, g0, g1, bc) in bn1_regions:
                    a0, a1 = max(g0, r0), min(g1, r1)
                    if a0 >= a1:
                        continue
                    nc.scalar.activation(
                        out=n1[p0:p1, a0:a1, P1:P1 + W],
                        in_=xst[p0:p1, a0 - r0:a1 - r0, :],
                        func=AF.Identity,
                        bias=col(bc, p0, p1),
                        scale=col("s1", p0, p1),
                    )

            # ---- depthwise conv stack ----
            # bsum accumulates the MIXER output directly: every branch's
            # H-taps (and all of c55) are folded into PE matmuls whose lhsT
            # is w11 composed with the per-channel tap (block-diag over s).
            bsum = P.tile([128, BR, W], BF16, tag="bs")

            def fold(b0, ntaps, rhs_fn, first):
                nb = BR // 2
                for g0 in range(0, nb, 6):
                    gs = list(range(g0, min(g0 + 6, nb)))
                    pss = [PS.tile([128, 2, W], F32, tag="ps",
                                   name=f"psf{b0}_{g0}_{j}")
                           for j in range(len(gs))]
                    for t in range(ntaps):
                        for j, k in enumerate(gs):
                            nc.tensor.matmul(
                                pss[j][:], blk(b0 + t), rhs_fn(t, k),
                                start=(t == 0), stop=(t == ntaps - 1))
                    for j, k in enumerate(gs):
                        dst = bsum[:, 2 * k:2 * k + 2, :]
                        if first:
                            nc.vector.tensor_copy(dst, pss[j][:])
                        else:
                            nc.vector.tensor_add(dst, dst, pss[j][:])

            # c55 (5x5 on n1) fully folded
            fold(TB_FOLD, 25,
                 lambda t, k: n1[:, 8 + t // 5 + 2 * k:10 + t // 5 + 2 * k,
                                 8 + t % 5:8 + t % 5 + W],
                 True)

            # cascaded branches: W-conv into u (with masked inner bias) on
            # DVE, then H-conv x w11 folded on PE
            u = P.tile([128, LR, W], BF16, tag="A")

            def wconv(nrows, h0, ntap, tapf, bias):
                """u[0:nrows] = sum_dw tap[dw]*n1[h0 + r, dw + (P1 - pad) + w] + bias"""
                pad = (ntap - 1) // 2
                # first tap with bias, split by boundary regions
                th = HALO - h0          # top halo rows in u coords
                bh = (SR - HALO) - 64 - h0  # = 75 - h0, bottom halo start
                regions = [
                    (0, 64, 0, th, bias + "t"),
                    (64, 128, 0, th, bias),
                    (0, 128, th, bh, bias),
                    (0, 64, bh, nrows, bias),
                    (64, 128, bh, nrows, bias + "b"),
                ]
                for (p0, p1, r0, r1, bc) in regions:
                    if r0 >= r1:
                        continue
                    nc.vector.tensor_scalar(
                        out=u[p0:p1, r0:r1, :],
                        in0=n1[p0:p1, h0 + r0:h0 + r1, P1 - pad:P1 - pad + W],
                        scalar1=col(tapf(0), p0, p1),
                        scalar2=col(bc, p0, p1),
                        op0=AO.mult, op1=AO.add)
                for dw in range(1, ntap):
                    nc.vector.scalar_tensor_tensor(
                        out=u[:, 0:nrows, :],
                        in0=n1[:, h0:h0 + nrows, P1 - pad + dw:P1 - pad + dw + W],
                        scalar=col(tapf(dw)),
                        in1=u[:, 0:nrows, :],
                        op0=AO.mult, op1=AO.add)

            wconv(72, 7, 7, lambda d: f"w17a_{d}", "b17a")
            fold(TB_F17, 7, lambda t, k: u[:, t + 2 * k:t + 2 * k + 2, :],
                 False)
            wconv(76, 5, 11, lambda d: f"w111a_{d}", "b111a")
            fold(TB_F111, 11, lambda t, k: u[:, t + 2 * k:t + 2 * k + 2, :],
                 False)
            wconv(86, 0, 21, lambda d: f"w211a_{d}", "b211a")
            fold(TB_F211, 21, lambda t, k: u[:, t + 2 * k:t + 2 * k + 2, :],
                 False)

            # ---- gating + layer-scale skip -> x_after ----
            xsk = P.tile([128, BR, W], F32, tag="A")
            nc.sync.dma_start(out=xsk[:], in_=x_d[:, RB - 1:RB - 1 + BR, :])
            for k in range(BR // 2):
                ps = PS.tile([128, 2, W], F32, tag="ps")
                # attn = (mix + b11') * n1   (psum as scratch)
                nc.vector.scalar_tensor_tensor(
                    out=ps[:], in0=bsum[:, 2 * k:2 * k + 2, :],
                    scalar=col("b11p"),
                    in1=n1[:, RB - 1 + 2 * k:RB + 1 + 2 * k, P1:P1 + W],
                    op0=AO.add, op1=AO.mult)
                # x_after = attn * ls1 + x
                nc.vector.scalar_tensor_tensor(
                    out=xsk[:, 2 * k:2 * k + 2, :], in0=ps[:],
                    scalar=col("ls1"), in1=xsk[:, 2 * k:2 * k + 2, :],
                    op0=AO.mult, op1=AO.add)

            # ---- n2 = BN2(x_after), boundary-masked ----
            n2 = P.tile([128, BR, NW], BF16, tag="n1")
            nc.gpsimd.memset(n2[:], 0.0)
            bn2_regions = [
                (0, 64, 0, 1, "t2top"),
                (0, 64, 1, BR, "t2"),
                (64, 128, 0, BR - 1, "t2"),
                (64, 128, BR - 1, BR, "t2bot"),
            ]
            for (p0, p1, r0, r1, bc) in bn2_regions:
                nc.scalar.activation(
                    out=n2[p0:p1, r0:r1, 1:1 + W],
                    in_=xsk[p0:p1, r0:r1, :],
                    func=AF.Identity,
                    bias=col(bc, p0, p1), scale=col("s2", p0, p1))

            # ---- FFN: fw1 (3x3-folded) -> gelu -> fw2 -> skip ----
            t3 = P.tile([128, 4, CH, W], BF16, tag="t3")
            nblk = CH // 2
            for cc in range(NCH):
                for ti in range(4):
                    psf = [PS.tile([128, 2, W], F32, tag="ps",
                                   name=f"ps1_{cc}_{ti}_{k}")
                           for k in range(nblk)]
                    for d in range(9):
                        dh, dw = d // 3, d % 3
                        for k in range(nblk):
                            row0 = cc * CH + 2 * k
                            nc.tensor.matmul(
                                psf[k][:], blk(TB_FW1 + ti * 9 + d),
                                n2[:, row0 + dh:row0 + dh + 2, dw:dw + W],
                                start=(d == 0), stop=(d == 8))
                    for k in range(nblk):
                        # gelu(psum + fb1') -> t3, with edge-row bias fixes
                        dst = t3[:, ti, 2 * k:2 * k + 2, :]
                        if cc == 0 and k == 0:
                            calls = [(0, 64, 0, 1, f"fb1e{ti}"),
                                     (64, 128, 0, 1, f"fb1p{ti}"),
                                     (0, 128, 1, 2, f"fb1p{ti}")]
                        elif cc == NCH - 1 and k == nblk - 1:
                            calls = [(0, 128, 0, 1, f"fb1p{ti}"),
                                     (0, 64, 1, 2, f"fb1p{ti}"),
                                     (64, 128, 1, 2, f"fb1f{ti}")]
                        else:
                            calls = [(0, 128, 0, 2, f"fb1p{ti}")]
                        for (p0, p1, r0, r1, bc) in calls:
                            nc.scalar.activation(
                                out=dst[p0:p1, r0:r1, :],
                                in_=psf[k][p0:p1, r0:r1, :],
                                func=AF.Gelu, bias=col(bc, p0, p1), scale=1.0)
                ost = P.tile([128, CH, W], F32, tag="bs")
                pso = [PS.tile([128, 2, W], F32, tag="ps",
                               name=f"ps2_{cc}_{k}")
                       for k in range(nblk)]
                for ti in range(4):
                    for k in range(nblk):
                        nc.tensor.matmul(
                            pso[k][:], blk(TB_FW2 + ti),
                            t3[:, ti, 2 * k:2 * k + 2, :],
                            start=(ti == 0), stop=(ti == 3))
                for k in range(nblk):
                    # y = psum * ls2 + ls2*fb2
                    nc.scalar.activation(
                        out=ost[:, 2 * k:2 * k + 2, :], in_=pso[k][:],
                        func=AF.Identity, bias=col("fb2p"), scale=col("ls2"))
                # += x_after rows
                nc.vector.tensor_add(
                    ost[:], ost[:], xsk[:, cc * CH + 1:cc * CH + 1 + CH, :])
                nc.sync.dma_start(out=o_d[:, cc * CH:cc * CH + CH, :],
                                  in_=ost[:])
    nc.compile()
    return nc


_NC_CACHE = None


def _get_nc():
    global _NC_CACHE
    if _NC_CACHE is None:
        _NC_CACHE = build_nc()
    return _NC_CACHE


# ---------------- host side ----------------
def _prep_core(inputs, b, half, params):
    """Per-core input dict."""
    x = inputs["x"]
    # x slice with halo rows, zero padded at image edges, arranged (s, c)
    r0 = 128 * half - HALO
    xs = np.zeros((2, C, LR, W), np.float32)
    for s in range(2):
        lo, hi = r0 + 64 * s, r0 + 64 * s + LR
        clo, chi = max(lo, 0), min(hi, 256)
        if clo < chi:
            xs[s, :, clo - lo:chi - lo, :] = x[b, :, clo:chi, :]
    cvec = params["cvec_top"] if half == 0 else params["cvec_bot"]
    return {"xs": xs.reshape(128, LR, W),
            "cvec": cvec, "tabs": params["tabs"]}


def _prep_params(inputs):
    ii = {k: np.asarray(v, np.float64) for k, v in inputs.items()}
    s1 = ii["g1"] / np.sqrt(ii["v1"] + EPS)
    t1 = ii["b1"] - ii["m1"] * s1
    s2 = ii["g2"] / np.sqrt(ii["v2"] + EPS)
    t2 = ii["b2"] - ii["m2"] * s2
    w55 = ii["w55"][:, 0]          # (C, 5, 5)
    w17a = ii["w17a"][:, 0, 0]     # (C, 7)
    w17b = ii["w17b"][:, 0, :, 0]  # (C, 7)
    w111a = ii["w111a"][:, 0, 0]
    w111b = ii["w111b"][:, 0, :, 0]
    w211a = ii["w211a"][:, 0, 0]
    w211b = ii["w211b"][:, 0, :, 0]
    w3 = ii["fdw"][:, 0]           # (HID, 3, 3)
    b0 = ii["bb55"] + ii["b17b"] + ii["b111b"] + ii["b211b"]
    b11p = ii["b11"] + ii["w11"] @ b0
    sall = w3.sum(axis=(1, 2))
    s_notop = w3[:, 1:, :].sum(axis=(1, 2))
    s_nobot = w3[:, :2, :].sum(axis=(1, 2))
    fb1p = ii["fbdw"] + ii["fb1"] * sall
    fb1e = ii["fbdw"] + ii["fb1"] * s_notop
    fb1f = ii["fbdw"] + ii["fb1"] * s_nobot

    def dup(v):
        return np.concatenate([v, v]).astype(np.float32)

    def cvec_for(half):
        cvb = np.zeros((128, NCOL), np.float32)

        def setc(name, v):
            cvb[:, _COLS[name]] = v

        top, bot = (half == 0), (half == 1)
        setc("s1", dup(s1)); setc("t1", dup(t1))
        setc("t1top", dup(t1 * (0.0 if top else 1.0)))
        setc("t1bot", dup(t1 * (0.0 if bot else 1.0)))
        setc("s2", dup(s2)); setc("t2", dup(t2))
        setc("t2top", dup(t2 * (0.0 if top else 1.0)))
        setc("t2bot", dup(t2 * (0.0 if bot else 1.0)))
        setc("b11p", dup(b11p)); setc("ls1", dup(ii["ls1"]))
        setc("ls2", dup(ii["ls2"])); setc("fb2p", dup(ii["ls2"] * ii["fb2"]))
        for nm, bb in (("b17a", ii["b17a"]), ("b111a", ii["b111a"]),
                       ("b211a", ii["b211a"])):
            setc(nm, dup(bb))
            setc(nm + "t", dup(bb * (0.0 if top else 1.0)))
            setc(nm + "b", dup(bb * (0.0 if bot else 1.0)))
        for t in range(4):
            j = slice(64 * t, 64 * t + 64)
            setc(f"fb1p{t}", dup(fb1p[j]))
            setc(f"fb1e{t}", dup(fb1e[j] if top else fb1p[j]))
            setc(f"fb1f{t}", dup(fb1f[j] if bot else fb1p[j]))
        for dh in range(5):
            for dw in range(5):
                setc(f"w55_{dh}_{dw}", dup(w55[:, dh, dw]))
        for d in range(7):
            setc(f"w17a_{d}", dup(w17a[:, d]))
            setc(f"w17b_{d}", dup(w17b[:, d]))
        for d in range(11):
            setc(f"w111a_{d}", dup(w111a[:, d]))
            setc(f"w111b_{d}", dup(w111b[:, d]))
        for d in range(21):
            setc(f"w211a_{d}", dup(w211a[:, d]))
            setc(f"w211b_{d}", dup(w211b[:, d]))
        return cvb

    # tabs: block-diagonal (over s) lhsT weight tables, bf16
    tabs = np.zeros((128, TBN), np.float64)
    fw1 = ii["fw1"]   # (HID, C)
    fw2 = ii["fw2"]   # (C, HID)
    w11 = ii["w11"]   # (C, C)

    def bd(m):  # (K, M) -> block diag over s: [(s,K), (s,M)]
        z = np.zeros((2 * m.shape[0], 2 * m.shape[1]))
        z[:m.shape[0], :m.shape[1]] = m
        z[m.shape[0]:, m.shape[1]:] = m
        return z

    # mixer-fold tables: lhsT[(s,c), (s,o)] = w11[o, c] * tap[c]
    w11T = w11.T                                          # (c, o)

    def setblk(i, m):
        tabs[:, i * 128:(i + 1) * 128] = bd(m)

    for dh in range(5):
        for dw in range(5):
            setblk(TB_FOLD + dh * 5 + dw, w11T * w55[:, dh, dw][:, None])
    for dh in range(7):
        setblk(TB_F17 + dh, w11T * w17b[:, dh][:, None])
    for dh in range(11):
        setblk(TB_F111 + dh, w11T * w111b[:, dh][:, None])
    for dh in range(21):
        setblk(TB_F211 + dh, w11T * w211b[:, dh][:, None])

    for ti in range(4):
        j = slice(64 * ti, 64 * ti + 64)
        for d in range(9):
            dh, dw = d // 3, d % 3
            # lhsT[(s,c), (s,j)] = fw1[j, c] * w3[j, dh, dw]
            m = (fw1[j, :] * w3[j, dh, dw][:, None]).T   # (C, 64)
            setblk(TB_FW1 + ti * 9 + d, m)
        m2 = fw2[:, j]                                    # (C, 64) -> (K=j, M=c)
        setblk(TB_FW2 + ti, m2.T)

    return {"cvec_top": cvec_for(0), "cvec_bot": cvec_for(1),
            "tabs": tabs.astype(BF)}


LAST_RESULTS = None


def _ensure_ntff_hook():
    """Recreate the missing antenv.axon_hooks registry and install the
    ctypes NTFF profiling hook (profiling-only; not needed to run)."""
    import sys
    import types
    try:
        from antenv.axon_hooks import get_axon_ntff_profile_hook  # noqa: F401
        return
    except ImportError:
        pass
    import antenv
    mod = types.ModuleType("antenv.axon_hooks")
    _hook_box = [None]
    mod.set_axon_ntff_profile_hook = lambda h: _hook_box.__setitem__(0, h)
    mod.get_axon_ntff_profile_hook = lambda: _hook_box[0]
    sys.modules["antenv.axon_hooks"] = mod
    antenv.axon_hooks = mod
    sys.path.insert(0, "/root/.axon_site/trn_agent_boot")
    try:
        import trn_boot
        hook = trn_boot._ntff_profile_via_ctypes("/opt/axon/libaxon_pjrt.so")
        mod.set_axon_ntff_profile_hook(hook)
    except Exception as e:  # pragma: no cover
        print("ntff hook install failed:", e)


def kernel(**inputs) -> np.ndarray:
    global LAST_RESULTS
    nc = _get_nc()
    params = _prep_params(inputs)
    in_maps = []
    for core in range(8):
        b, half = core // 2, core % 2
        in_maps.append(_prep_core(inputs, b, half, params))
    import os
    trace = bool(int(os.environ.get("KTRACE", "0")))
    if trace:
        _ensure_ntff_hook()
    res = run_bass_kernel_spmd(nc, in_maps, core_ids=list(range(8)),
                               trace=trace)
    LAST_RESULTS = res
    out = np.zeros((4, C, 256, W), np.float32)
    for core in range(8):
        b, half = core // 2, core % 2
        o = res.results[core]["out"].reshape(2, C, 64, W)
        for s in range(2):
            r = 128 * half + 64 * s
            out[b, :, r:r + 64, :] = o[s]
    return out
